# revision 19
# baseline (speedup 1.0000x reference)
"""Causal MHA (B=2, N=2048, D=1024, H=16) on 8 NeuronCores via Bass/Tile.

Sharding: core c = (b, g) with b = c // 4 (batch), g = c % 4 (head group of 4
heads = 256 features). Each core computes Q/K/V projections for its feature
slice, causal attention for its 4 heads, and a partial output projection
(its 256 rows of Wo). Host sums the 4 partials per batch.

Layout: everything is computed "feature-major" (features on SBUF partitions,
sequence on the free axis). S^T = K Q^T tiles come out of the PE with k on
partitions and q on the free axis, so exp() needs no reduction; the softmax
denominator comes from augmenting V with a ones column in the P@V matmul, and
the per-(head, q) normalization uses a PE-broadcast reciprocal row. Two heads
are packed per ST step (row groups 0-63 / 64-127 run concurrently in the PE
array) sharing one 2-bank PSUM tile and a single fused exp. All matmuls run
in float32r (~1.5e-4 rel err, full PE rate at free-dim >= 256).
"""

import numpy as np

import concourse.bacc as bacc
import concourse.mybir as mybir
from concourse.tile import TileContext
from concourse.bass_utils import run_bass_kernel_spmd

F32 = mybir.dt.float32
F32R = mybir.dt.float32r
AF = mybir.ActivationFunctionType

B, N, D, H, DH = 2, 2048, 1024, 16, 64
NCORES = 8
GROUPS = 4            # head groups (cores per batch)
HPC = H // GROUPS     # 4 heads per core
FS = HPC * DH         # 256-feature slice per core
P = 128
NDT = N // 128        # 16 seq tiles of 128
NSS = N // 512        # 4 seq slices of 512
DT = D // 128         # 8 d tiles
FT = FS // 128        # 2 feature tiles per core

_CACHE = {}


def _build(repeat=1):
    nc = bacc.Bacc("TRN2", target_bir_lowering=False, debug=False)

    xqT = nc.dram_tensor("xqT", [D, N], F32R, kind="ExternalInput")
    xkvT = nc.dram_tensor("xkvT", [D, N], F32R, kind="ExternalInput")
    wq = nc.dram_tensor("wq", [D, FS], F32R, kind="ExternalInput")
    wk = nc.dram_tensor("wk", [D, FS], F32R, kind="ExternalInput")
    wv = nc.dram_tensor("wv", [D, FS], F32R, kind="ExternalInput")
    wo = nc.dram_tensor("wo", [FS, D], F32R, kind="ExternalInput")
    bq = nc.dram_tensor("bq", [FS], F32, kind="ExternalInput")
    bk = nc.dram_tensor("bk", [FS], F32, kind="ExternalInput")
    bv = nc.dram_tensor("bv", [1, FS], F32R, kind="ExternalInput")
    bo = nc.dram_tensor("bo", [1, D], F32R, kind="ExternalInput")
    masks = nc.dram_tensor("masks", [4, P, 512], F32R, kind="ExternalInput")
    out = nc.dram_tensor("out_p", [N, D], F32, kind="ExternalOutput")

    with TileContext(nc) as tc:
        with (
            tc.tile_pool(name="const", bufs=1) as cp,
            tc.tile_pool(name="xt", bufs=1) as xp,
            tc.tile_pool(name="acts", bufs=1) as ap_,
            tc.tile_pool(name="ps", bufs=4, space="PSUM") as psp,
            tc.tile_pool(name="pt", bufs=3) as ptp,
            tc.tile_pool(name="small", bufs=4) as smp,
            tc.tile_pool(name="osb", bufs=2) as osp,
        ):
            w_bufs = 2 if repeat == 1 else 3
            wk_sb = cp.tile([P, DT, FS], F32R, tag="w", bufs=w_bufs, name="wk_sb")
            wv_sb = cp.tile([P, DT, FS], F32R, tag="w", bufs=w_bufs, name="wv_sb")
            wo_sb = cp.tile([P, FT, D], F32R, tag="wo")
            bqk_sb = cp.tile([P, 2, 2], F32, tag="bqk")
            bv_sb = cp.tile([1, FS], F32R, tag="bv")
            bo_sb = cp.tile([1, D], F32R, tag="bo")
            mask_sb = cp.tile([P, 4, 512], F32R, tag="mask")
            ones_r = cp.tile([1, P], F32R, tag="ones")
            ones_f = cp.tile([P, HPC], F32, tag="ones_f")
            ones_fr = cp.tile([1, P], F32, tag="ones_fr")

            nc.sync.dma_start(out=wk_sb, in_=wk.ap().rearrange("(t p) f -> p t f", p=P))
            nc.sync.dma_start(out=wv_sb, in_=wv.ap().rearrange("(t p) f -> p t f", p=P))
            nc.sync.dma_start(out=wo_sb, in_=wo.ap().rearrange("(t p) f -> p t f", p=P))
            nc.sync.dma_start(out=bqk_sb[:, 0, :], in_=bk.ap().rearrange("(t p) -> p t", p=P))
            nc.sync.dma_start(out=bqk_sb[:, 1, :], in_=bq.ap().rearrange("(t p) -> p t", p=P))
            nc.sync.dma_start(out=bv_sb, in_=bv.ap())
            nc.sync.dma_start(out=bo_sb, in_=bo.ap())
            nc.sync.dma_start(out=mask_sb, in_=masks.ap().rearrange("k p f -> p k f"))
            nc.vector.memset(ones_f, 1.0)
            nc.vector.memset(ones_fr, 1.0)
            nc.vector.tensor_copy(ones_r, ones_fr)

            kt_all = [ap_.tile([P, N], F32R, tag=f"kt{f}", name=f"kt{f}") for f in range(FT)]
            qt_all = [ap_.tile([P, N], F32R, tag=f"qt{f}", name=f"qt{f}") for f in range(FT)]
            v_sb = [ap_.tile([P, HPC, DH + 1], F32R, tag=f"v{st}", name=f"v{st}") for st in range(NDT)]
            ot_all = [ap_.tile([P, N], F32R, tag=f"ot{f}", name=f"ot{f}") for f in range(FT)]

            def emit_body():
                # ---- load x_kv (transposed layout straight from DRAM) ----
                xkv_t = []
                for d in range(DT):
                    t = xp.tile([P, N], F32R, tag=f"x{d}", name=f"xkvt{d}")
                    nc.sync.dma_start(out=t, in_=xkvT.ap()[d * P:(d + 1) * P, :])
                    xkv_t.append(t)

                # ---- K projection (feature-major) ----
                for ft in range(FT):
                    for ss in range(NSS):
                        ps = psp.tile([P, 512], F32, tag="ps", name="ps_k")
                        for d in range(DT):
                            nc.tensor.matmul(
                                ps,
                                wk_sb[:, d, ft * P:(ft + 1) * P],
                                xkv_t[d][:, ss * 512:(ss + 1) * 512],
                                start=(d == 0),
                                stop=(d == DT - 1),
                            )
                        nc.scalar.activation(
                            kt_all[ft][:, ss * 512:(ss + 1) * 512],
                            ps, AF.Identity, bias=bqk_sb[:, 0, ft:ft + 1],
                        )

                # ---- V projection (natural layout, ones column appended) ----
                for st in range(NDT):
                    psv = psp.tile([P, 512], F32, tag="ps", name="ps_v")
                    for d in range(DT):
                        nc.tensor.matmul(
                            psv[:, 0:FS],
                            xkv_t[d][:, st * P:(st + 1) * P],
                            wv_sb[:, d, :],
                            start=(d == 0),
                            stop=False,
                        )
                    nc.tensor.matmul(psv[:, 0:FS], ones_r[:, 0:P], bv_sb, start=False, stop=True)
                    nc.vector.tensor_copy(
                        v_sb[st][:, :, 0:DH],
                        psv[:, 0:FS].rearrange("p (h c) -> p h c", h=HPC),
                    )
                    nc.vector.tensor_copy(v_sb[st][:, :, DH], ones_f)

                # ---- Q projection (reuses xt slots; wq reuses wk's slot) ----
                wq_sb = cp.tile([P, DT, FS], F32R, tag="w", bufs=w_bufs, name="wq_sb")
                nc.sync.dma_start(out=wq_sb, in_=wq.ap().rearrange("(t p) f -> p t f", p=P))
                xq_t = []
                for d in range(DT):
                    t = xp.tile([P, N], F32R, tag=f"x{d}", name=f"xqt{d}")
                    nc.sync.dma_start(out=t, in_=xqT.ap()[d * P:(d + 1) * P, :])
                    xq_t.append(t)
                for ft in range(FT):
                    for ss in range(NSS):
                        ps = psp.tile([P, 512], F32, tag="ps", name="ps_q")
                        for d in range(DT):
                            nc.tensor.matmul(
                                ps,
                                wq_sb[:, d, ft * P:(ft + 1) * P],
                                xq_t[d][:, ss * 512:(ss + 1) * 512],
                                start=(d == 0),
                                stop=(d == DT - 1),
                            )
                        nc.scalar.activation(
                            qt_all[ft][:, ss * 512:(ss + 1) * 512],
                            ps, AF.Identity, bias=bqk_sb[:, 1, ft:ft + 1],
                        )

                # ---- attention: 2 heads packed per ST step via PE row groups ----
                for ft in range(FT):
                    for ss in range(NSS):
                        otp = [
                            psp.tile([P, 512], F32, tag="ps", name=f"ps_ot{hh}")
                            for hh in range(2)
                        ]
                        n_kt = 4 * ss + 4
                        for kt in range(n_kt):
                            st2 = psp.tile([P, 1024], F32, tag="ps2", bufs=2, name="ps_st2")
                            ptt = ptp.tile([P, 1024], F32R, tag="pt", name="ptt")
                            for hh in range(2):
                                nc.tensor.matmul(
                                    st2[:, hh * 512:(hh + 1) * 512],
                                    kt_all[ft][hh * 64:(hh + 1) * 64, kt * P:(kt + 1) * P],
                                    qt_all[ft][hh * 64:(hh + 1) * 64, ss * 512:(ss + 1) * 512],
                                    start=True, stop=True,
                                )
                            nc.scalar.activation(ptt, st2, AF.Exp, scale=0.125)
                            dk = kt - 4 * ss
                            if dk >= 0:
                                for hh in range(2):
                                    nc.vector.tensor_mul(
                                        ptt[:, hh * 512:(hh + 1) * 512],
                                        ptt[:, hh * 512:(hh + 1) * 512],
                                        mask_sb[:, dk, :],
                                    )
                            for hh in range(2):
                                nc.tensor.matmul(
                                    otp[hh][0:DH + 1, :],
                                    v_sb[kt][:, ft * 2 + hh, :],
                                    ptt[:, hh * 512:(hh + 1) * 512],
                                    start=(kt == 0),
                                    stop=(kt == n_kt - 1),
                                )
                        for hh in range(2):
                            row = hh * 64
                            recip = smp.tile([1, 512], F32R, tag="recip", name="recip")
                            with nc.allow_low_precision(reason="f32r recip feeds f32r broadcast matmul"):
                                nc.vector.reciprocal(recip, otp[hh][DH:DH + 1, :])
                            rep_ps = psp.tile([P, 512], F32, tag="ps", name="ps_rep")
                            nc.tensor.matmul(rep_ps[0:DH, :], ones_r[:, 0:DH], recip, start=True, stop=True)
                            rep_sb = smp.tile([DH, 512], F32, tag="rep_sb", bufs=2, name="rep_sb")
                            nc.vector.tensor_copy(rep_sb, rep_ps[0:DH, :])
                            nc.vector.tensor_mul(
                                ot_all[ft][row:row + 64, ss * 512:(ss + 1) * 512],
                                otp[hh][0:DH, :],
                                rep_sb,
                            )

                # ---- output projection + bias, partial over this core's features ----
                for qt in range(NDT):
                    o_sb = osp.tile([P, D], F32, tag="osb", name="o_sb")
                    for os_ in range(2):
                        ps_o = psp.tile([P, 512], F32, tag="ps", name="ps_o")
                        for ft in range(FT):
                            nc.tensor.matmul(
                                ps_o,
                                ot_all[ft][:, qt * P:(qt + 1) * P],
                                wo_sb[:, ft, os_ * 512:(os_ + 1) * 512],
                                start=(ft == 0),
                                stop=False,
                            )
                        nc.tensor.matmul(
                            ps_o, ones_r[:, 0:P], bo_sb[:, os_ * 512:(os_ + 1) * 512],
                            start=False, stop=True,
                        )
                        nc.vector.tensor_copy(o_sb[:, os_ * 512:(os_ + 1) * 512], ps_o)
                    nc.sync.dma_start(out=out.ap()[qt * P:(qt + 1) * P, :], in_=o_sb)

            if repeat == 1:
                emit_body()
            else:
                with tc.For_i(0, repeat, 1):
                    emit_body()

    nc.compile()
    return nc


def _shard_inputs(x_q, x_kv, Wq, bq_, Wk, bk_, Wv, bv_, Wo, bo_):
    """Build the 8 per-core input maps."""
    mask = np.zeros((4, P, 512), dtype=np.float32)
    pp_, ff = np.meshgrid(np.arange(P), np.arange(512), indexing="ij")
    for i, dk in enumerate((0, 128, 256, 384)):
        mask[i] = (ff >= pp_ + dk).astype(np.float32)
    in_maps = []
    for c in range(NCORES):
        b, g = c // GROUPS, c % GROUPS
        sl = slice(g * FS, (g + 1) * FS)
        in_maps.append({
            "xqT": np.ascontiguousarray(x_q[b].T),
            "xkvT": np.ascontiguousarray(x_kv[b].T),
            "wq": np.ascontiguousarray(Wq[:, sl]),
            "wk": np.ascontiguousarray(Wk[:, sl]),
            "wv": np.ascontiguousarray(Wv[:, sl]),
            "wo": np.ascontiguousarray(Wo[sl, :]),
            "bq": np.ascontiguousarray(bq_[sl]),
            "bk": np.ascontiguousarray(bk_[sl]),
            "bv": np.ascontiguousarray(bv_[sl]).reshape(1, FS),
            "bo": (bo_ if g == 0 else np.zeros_like(bo_)).reshape(1, D),
            "masks": mask,
        })
    return in_maps


def kernel(x_q, x_kv, Wq, bq, Wk, bk, Wv, bv, Wo, bo):
    x_q = np.asarray(x_q, dtype=np.float32)
    x_kv = np.asarray(x_kv, dtype=np.float32)
    if "nc" not in _CACHE:
        _CACHE["nc"] = _build()
    nc = _CACHE["nc"]
    in_maps = _shard_inputs(
        x_q, x_kv,
        np.asarray(Wq, np.float32), np.asarray(bq, np.float32),
        np.asarray(Wk, np.float32), np.asarray(bk, np.float32),
        np.asarray(Wv, np.float32), np.asarray(bv, np.float32),
        np.asarray(Wo, np.float32), np.asarray(bo, np.float32),
    )
    res = run_bass_kernel_spmd(nc, in_maps, core_ids=list(range(NCORES)))
    out = np.zeros((B, N, D), dtype=np.float32)
    for c in range(NCORES):
        out[c // GROUPS] += res.results[c]["out_p"]
    return out


# revision 31
# speedup vs baseline: 1.0001x; 1.0001x over previous
"""Causal MHA (B=2, N=2048, D=1024, H=16) on 8 NeuronCores via Bass/Tile.

Sharding: core c = (b, g): b = c // 4 (batch), g = c % 4 (head group of 4
heads = 256 features). Each core computes its Q/K/V projections, causal
attention for its 4 heads, and a partial output projection (its 256 rows of
Wo). The host sums the 4 partials per batch ("unshard" of row-parallel TP).

Layout: activations are feature-major (features on SBUF partitions, sequence
on the free axis), so S^T = K Q^T tiles come out of the PE with k on
partitions and q free and exp() needs no reduction at all. The softmax
denominator falls out of the P@V matmul via a ones column appended to V; the
per-(head, q) normalization uses a reciprocal row broadcast across partitions
through a DRAM bounce. Projections stream x in two d-halves (SBUF partial
sums) so x_q loads overlap the x_kv passes; the output projection is
interleaved into the attention ss loop so PE/DMA stay busy end-to-end.
All matmuls run in float32r (~1.5e-4 rel err, full PE rate at free >= 256).
"""

import numpy as np

import concourse.bass as bass
import concourse.bacc as bacc
import concourse.mybir as mybir
from concourse.tile import TileContext
from concourse.bass_utils import run_bass_kernel_spmd

F32 = mybir.dt.float32
F32R = mybir.dt.float32r
AF = mybir.ActivationFunctionType

B, N, D, H, DH = 2, 2048, 1024, 16, 64
NCORES = 8
GROUPS = 4
HPC = H // GROUPS     # 4 heads per core
FS = HPC * DH         # 256
P = 128
NDT = N // 128        # 16
NSS = N // 512        # 4
DT = D // 128         # 8
FT = FS // 128        # 2
DH2 = DT // 2         # d-tiles per half

_CACHE = {}


def _build(repeat=1, phases="all"):
    nc = bacc.Bacc("TRN2", target_bir_lowering=False, debug=False)

    xqT = nc.dram_tensor("xqT", [D, N], F32R, kind="ExternalInput")
    xkvT = nc.dram_tensor("xkvT", [D, N], F32R, kind="ExternalInput")
    wq = nc.dram_tensor("wq", [D, FS], F32R, kind="ExternalInput")
    wk = nc.dram_tensor("wk", [D, FS], F32R, kind="ExternalInput")
    wv = nc.dram_tensor("wv", [D, FS], F32R, kind="ExternalInput")
    wo = nc.dram_tensor("wo", [FS, D], F32R, kind="ExternalInput")
    bq = nc.dram_tensor("bq", [FS], F32, kind="ExternalInput")
    bk = nc.dram_tensor("bk", [FS], F32, kind="ExternalInput")
    bv = nc.dram_tensor("bv", [1, FS], F32R, kind="ExternalInput")
    bo = nc.dram_tensor("bo", [1, D], F32R, kind="ExternalInput")
    masks = nc.dram_tensor("masks", [P, P], F32R, kind="ExternalInput")
    out = nc.dram_tensor("out_p", [N, D], F32, kind="ExternalOutput")

    with TileContext(nc) as tc:
        with (
            tc.tile_pool(name="const", bufs=1) as cp,
            tc.tile_pool(name="xt", bufs=1) as xp,
            tc.tile_pool(name="acts", bufs=1) as ap_,
            tc.tile_pool(name="ps", bufs=4, space="PSUM") as psp,
            tc.tile_pool(name="pt", bufs=2) as ptp,
            tc.tile_pool(name="small", bufs=4) as smp,
            tc.tile_pool(name="osb", bufs=2) as osp,
            tc.tile_pool(name="dsc", bufs=4, space="DRAM") as dsp,
        ):
            wo_sb = cp.tile([P, FT, D], F32R, tag="wo")
            bqk_sb = cp.tile([P, 2, 2], F32, tag="bqk")
            bv_sb = cp.tile([1, FS], F32R, tag="bv")
            bo_sb = cp.tile([1, D], F32R, tag="bo")
            tri_sb = cp.tile([P, P], F32R, tag="mask")
            ones_r = cp.tile([1, P], F32R, tag="ones")
            ones_f = cp.tile([P, HPC], F32, tag="ones_f")
            ones_fr = cp.tile([1, P], F32, tag="ones_fr")
            bo_rep = cp.tile([P, D], F32, tag="bo_rep")
            bv_rep = cp.tile([P, FS], F32, tag="bv_rep")

            nc.sync.dma_start(out=wo_sb, in_=wo.ap().rearrange("(t p) f -> p t f", p=P))
            nc.sync.dma_start(out=bqk_sb[:, 0, :], in_=bk.ap().rearrange("(t p) -> p t", p=P))
            nc.sync.dma_start(out=bqk_sb[:, 1, :], in_=bq.ap().rearrange("(t p) -> p t", p=P))
            nc.sync.dma_start(out=bv_sb, in_=bv.ap())
            nc.sync.dma_start(out=bo_sb, in_=bo.ap())
            nc.sync.dma_start(out=tri_sb, in_=masks.ap())
            nc.vector.memset(ones_f, 1.0)
            nc.vector.memset(ones_fr, 1.0)
            nc.vector.tensor_copy(ones_r, ones_fr)

            # one-time replicated bias tiles (replaces per-tile K=1 matmuls,
            # which measure ~1.1us each on HW)
            ps_rep = psp.tile([P, 512], F32, tag="ps", name="ps_brep")
            nc.tensor.matmul(ps_rep, ones_r[:, 0:P], bo_sb[:, 0:512], start=True, stop=True)
            nc.vector.tensor_copy(bo_rep[:, 0:512], ps_rep)
            ps_rep2 = psp.tile([P, 512], F32, tag="ps", name="ps_brep2")
            nc.tensor.matmul(ps_rep2, ones_r[:, 0:P], bo_sb[:, 512:1024], start=True, stop=True)
            nc.vector.tensor_copy(bo_rep[:, 512:1024], ps_rep2)
            ps_rep3 = psp.tile([P, 512], F32, tag="ps", name="ps_brep3")
            nc.tensor.matmul(ps_rep3[:, 0:FS], ones_r[:, 0:P], bv_sb, start=True, stop=True)
            nc.vector.tensor_copy(bv_rep, ps_rep3[:, 0:FS])

            kt_all = [ap_.tile([P, N], F32R, tag=f"kt{f}", name=f"kt{f}") for f in range(FT)]
            qt_all = [ap_.tile([P, N], F32R, tag=f"qt{f}", name=f"qt{f}") for f in range(FT)]
            v_sb = [ap_.tile([P, HPC, DH + 1], F32R, tag=f"v{st}", name=f"v{st}") for st in range(NDT)]
            ot_all = [ap_.tile([P, N], F32R, tag=f"ot{f}", name=f"ot{f}") for f in range(FT)]

            def emit_body():
                # ---- projections, streamed in two d-halves ----
                for half in range(2):
                    d0 = half * DH2
                    wk_sb = cp.tile([P, DH2, FS], F32R, tag="w", bufs=2, name="wk_h")
                    nc.sync.dma_start(out=wk_sb, in_=wk.ap().rearrange("(t p) f -> p t f", p=P)[:, d0:d0 + DH2, :])
                    wv_sb = cp.tile([P, DH2, FS], F32R, tag="w", bufs=2, name="wv_h")
                    nc.sync.dma_start(out=wv_sb, in_=wv.ap().rearrange("(t p) f -> p t f", p=P)[:, d0:d0 + DH2, :])
                    wq_sb = cp.tile([P, DH2, FS], F32R, tag="w", bufs=2, name="wq_h")
                    nc.sync.dma_start(out=wq_sb, in_=wq.ap().rearrange("(t p) f -> p t f", p=P)[:, d0:d0 + DH2, :])
                    xkv_t, xq_t = [], []
                    for i in range(DH2):
                        d = d0 + i
                        t = xp.tile([P, N], F32R, tag=f"xkv{i}", name=f"xkv{i}")
                        nc.sync.dma_start(out=t, in_=xkvT.ap()[d * P:(d + 1) * P, :])
                        xkv_t.append(t)
                    for i in range(DH2):
                        d = d0 + i
                        t = xp.tile([P, N], F32R, tag=f"xq{i}", name=f"xq{i}")
                        nc.sync.dma_start(out=t, in_=xqT.ap()[d * P:(d + 1) * P, :])
                        xq_t.append(t)

                    # K pass
                    for ft in range(FT):
                        for ss in range(NSS):
                            ps = psp.tile([P, 512], F32, tag="ps", name="ps_k")
                            for i in range(DH2):
                                nc.tensor.matmul(
                                    ps,
                                    wk_sb[:, i, ft * P:(ft + 1) * P],
                                    xkv_t[i][:, ss * 512:(ss + 1) * 512],
                                    start=(i == 0),
                                    stop=(i == DH2 - 1),
                                )
                            dst = kt_all[ft][:, ss * 512:(ss + 1) * 512]
                            if half == 0:
                                nc.scalar.activation(dst, ps, AF.Identity, bias=bqk_sb[:, 0, ft:ft + 1])
                            else:
                                nc.vector.tensor_add(dst, dst, ps)
                    # V pass
                    for st in range(NDT):
                        psv = psp.tile([P, 512], F32, tag="ps", name="ps_v")
                        for i in range(DH2):
                            nc.tensor.matmul(
                                psv[:, 0:FS],
                                xkv_t[i][:, st * P:(st + 1) * P],
                                wv_sb[:, i, :],
                                start=(i == 0),
                                stop=(i == DH2 - 1),
                            )
                        vdst = v_sb[st][:, :, 0:DH]
                        psv_v = psv[:, 0:FS].rearrange("p (h c) -> p h c", h=HPC)
                        if half == 0:
                            nc.vector.tensor_add(vdst, psv_v, bv_rep.rearrange("p (h c) -> p h c", h=HPC))
                            nc.vector.tensor_copy(v_sb[st][:, :, DH], ones_f)
                        else:
                            nc.vector.tensor_add(vdst, vdst, psv_v)
                    # Q pass
                    for ft in range(FT):
                        for ss in range(NSS):
                            ps = psp.tile([P, 512], F32, tag="ps", name="ps_q")
                            for i in range(DH2):
                                nc.tensor.matmul(
                                    ps,
                                    wq_sb[:, i, ft * P:(ft + 1) * P],
                                    xq_t[i][:, ss * 512:(ss + 1) * 512],
                                    start=(i == 0),
                                    stop=(i == DH2 - 1),
                                )
                            dst = qt_all[ft][:, ss * 512:(ss + 1) * 512]
                            if half == 0:
                                nc.scalar.activation(dst, ps, AF.Identity, bias=bqk_sb[:, 1, ft:ft + 1])
                            else:
                                nc.vector.tensor_add(dst, dst, ps)

                if phases == "proj":
                    row = 0
                    for tset in (kt_all, qt_all):
                        for tt in tset:
                            for half in range(2):
                                nc.sync.dma_start(
                                    out=out.ap()[row * P:(row + 1) * P, :],
                                    in_=tt[:, half * D:(half + 1) * D].bitcast(F32),
                                )
                                row += 1
                    for st in range(NDT):
                        rr = 8 + st % 8
                        nc.sync.dma_start(
                            out=out.ap()[rr * P:(rr + 1) * P, 0:HPC * (DH + 1)],
                            in_=v_sb[st].rearrange("p h c -> p (h c)").bitcast(F32),
                        )
                    return

                # ---- attention (2 heads packed per ST step) + interleaved O-proj ----
                for ss in range(NSS):
                    n_kt = 4 * ss + 4
                    for ft in range(FT):
                        otp = [
                            psp.tile([P, 512], F32, tag="ps", name=f"ps_ot{hh}")
                            for hh in range(2)
                        ]
                        for kt in range(n_kt):
                            st2 = psp.tile([P, 1024], F32, tag="ps2", bufs=2, name="ps_st2")
                            ptt = ptp.tile([P, 1024], F32R, tag="pt", name="ptt")
                            for hh in range(2):
                                nc.tensor.matmul(
                                    st2[:, hh * 512:(hh + 1) * 512],
                                    kt_all[ft][hh * 64:(hh + 1) * 64, kt * P:(kt + 1) * P],
                                    qt_all[ft][hh * 64:(hh + 1) * 64, ss * 512:(ss + 1) * 512],
                                    start=True, stop=True,
                                )
                            nc.scalar.activation(ptt, st2, AF.Exp, scale=0.125)
                            dk = (kt - 4 * ss) * P
                            if dk >= 0:
                                for hh in range(2):
                                    base = hh * 512
                                    if dk > 0:
                                        nc.vector.tensor_scalar_mul(
                                            ptt[:, base:base + dk],
                                            ptt[:, base:base + dk],
                                            0.0,
                                        )
                                    nc.vector.tensor_mul(
                                        ptt[:, base + dk:base + dk + P],
                                        ptt[:, base + dk:base + dk + P],
                                        tri_sb,
                                    )
                            for hh in range(2):
                                nc.tensor.matmul(
                                    otp[hh][0:DH + 1, :],
                                    v_sb[kt][:, ft * 2 + hh, :],
                                    ptt[:, hh * 512:(hh + 1) * 512],
                                    start=(kt == 0),
                                    stop=(kt == n_kt - 1),
                                )
                        # normalization: reciprocal row, broadcast via DRAM bounce
                        rept = smp.tile([DH + 1, 1024], F32R, tag="rep_sb", bufs=2, name="rept")
                        recip = rept[DH:DH + 1, :]
                        rep_sb = rept[0:DH, :]
                        with nc.allow_low_precision(reason="softmax reciprocal"):
                            nc.vector.reciprocal(recip[:, 0:512], otp[0][DH:DH + 1, :])
                            nc.vector.reciprocal(recip[:, 512:1024], otp[1][DH:DH + 1, :])
                        dscr = dsp.tile([1, 1024], F32R, tag="dscr", name="dscr")
                        nc.sync.dma_start(out=dscr, in_=recip)
                        rep_bcast = bass.AP(
                            tensor=dscr.tensor,
                            offset=dscr.offset,
                            ap=[[0, DH]] + [list(x) for x in dscr.ap[1:]],
                        )
                        nc.sync.dma_start(out=rep_sb, in_=rep_bcast)
                        for hh in range(2):
                            row = hh * 64
                            nc.vector.tensor_mul(
                                ot_all[ft][row:row + 64, ss * 512:(ss + 1) * 512],
                                otp[hh][0:DH, :],
                                rep_sb[:, hh * 512:(hh + 1) * 512],
                            )

                    if phases == "proj+attn":
                        continue
                    # O-proj for the q-tiles of this ss slice
                    for qt in range(4 * ss, 4 * ss + 4):
                        o_sb = osp.tile([P, D], F32, tag="osb", name="o_sb")
                        for os_ in range(2):
                            ps_o = psp.tile([P, 512], F32, tag="ps", name="ps_o")
                            for ft in range(FT):
                                nc.tensor.matmul(
                                    ps_o,
                                    ot_all[ft][:, qt * P:(qt + 1) * P],
                                    wo_sb[:, ft, os_ * 512:(os_ + 1) * 512],
                                    start=(ft == 0),
                                    stop=(ft == FT - 1),
                                )
                            nc.vector.tensor_add(
                                o_sb[:, os_ * 512:(os_ + 1) * 512],
                                ps_o,
                                bo_rep[:, os_ * 512:(os_ + 1) * 512],
                            )
                        nc.sync.dma_start(out=out.ap()[qt * P:(qt + 1) * P, :], in_=o_sb)

                if phases == "proj+attn":
                    row = 0
                    for tt in ot_all:
                        for half in range(2):
                            nc.sync.dma_start(
                                out=out.ap()[row * P:(row + 1) * P, :],
                                in_=tt[:, half * D:(half + 1) * D].bitcast(F32),
                            )
                            row += 1
                    return

            if repeat == 1:
                emit_body()
            else:
                with tc.For_i(0, repeat, 1):
                    emit_body()

    nc.compile()
    return nc


def _shard_inputs(x_q, x_kv, Wq, bq_, Wk, bk_, Wv, bv_, Wo, bo_):
    pp_, ff = np.meshgrid(np.arange(P), np.arange(P), indexing="ij")
    mask = (ff >= pp_).astype(np.float32)
    in_maps = []
    for c in range(NCORES):
        b, g = c // GROUPS, c % GROUPS
        sl = slice(g * FS, (g + 1) * FS)
        in_maps.append({
            "xqT": np.ascontiguousarray(x_q[b].T),
            "xkvT": np.ascontiguousarray(x_kv[b].T),
            "wq": np.ascontiguousarray(Wq[:, sl]),
            "wk": np.ascontiguousarray(Wk[:, sl]),
            "wv": np.ascontiguousarray(Wv[:, sl]),
            "wo": np.ascontiguousarray(Wo[sl, :]),
            "bq": np.ascontiguousarray(bq_[sl]),
            "bk": np.ascontiguousarray(bk_[sl]),
            "bv": np.ascontiguousarray(bv_[sl]).reshape(1, FS),
            "bo": (bo_ if g == 0 else np.zeros_like(bo_)).reshape(1, D),
            "masks": mask,
        })
    return in_maps


def kernel(x_q, x_kv, Wq, bq, Wk, bk, Wv, bv, Wo, bo):
    x_q = np.asarray(x_q, dtype=np.float32)
    x_kv = np.asarray(x_kv, dtype=np.float32)
    if "nc" not in _CACHE:
        _CACHE["nc"] = _build()
    nc = _CACHE["nc"]
    in_maps = _shard_inputs(
        x_q, x_kv,
        np.asarray(Wq, np.float32), np.asarray(bq, np.float32),
        np.asarray(Wk, np.float32), np.asarray(bk, np.float32),
        np.asarray(Wv, np.float32), np.asarray(bv, np.float32),
        np.asarray(Wo, np.float32), np.asarray(bo, np.float32),
    )
    res = run_bass_kernel_spmd(nc, in_maps, core_ids=list(range(NCORES)))
    out = np.zeros((B, N, D), dtype=np.float32)
    for c in range(NCORES):
        out[c // GROUPS] += res.results[c]["out_p"]
    return out
